# revision 21
# baseline (speedup 1.0000x reference)
"""ArcFace loss on 8 TRN2 NeuronCores, sharded along the class dim C.

Per core: stream the [512, 12500] cosine shard through ScalarE
exp(64*x - 64) with per-row accumulation (one HBM pass), gather the
per-row target element via indirect DMA, apply the angular margin on
tiny [128,4] tensors, AllReduce the per-row (sumexp, target-logit)
stats, then loss = 64 + mean(log(sumexp) - tgt).

A zero-valued warmup AllReduce is issued first so the one-time
collective/comm alignment overlaps the streaming pass instead of
serializing after it.
"""

import math
import os

import numpy as np

import concourse.bacc as bacc
import concourse.bass as bass
import concourse.bass_isa as bass_isa
import concourse.mybir as mybir
import concourse.tile as tile
from concourse.bass_utils import run_bass_kernel_spmd

# ArcFace constants (match the reference)
S = 64.0
M = 0.5
COS_M = math.cos(M)
SIN_M = math.sin(M)
TH = math.cos(math.pi - M)
MM = math.sin(math.pi - M) * M
EPS = 1e-07

B, C = 512, 100000
NCORES = 8
CS = C // NCORES  # 12500 classes per core
P = 128
RT = B // P  # 4 row tiles
FC = int(os.environ.get("K_FC", "6250"))  # steady-state chunk width
# row-tile 0 ramps up so the ACT pipeline starts early
RAMP_PLAN = [625, 625, 1250, 2500, 3750, 3750]
NCH = CS // FC
SHIFT = 64.0  # exp(S*c - SHIFT) keeps everything <= 1 since c in [-1, 1]

F32 = mybir.dt.float32
I32 = mybir.dt.int32


WARMUP_CC = os.environ.get("K_WARMUP", "0") == "1"
CC_KIND = os.environ.get("K_CC", "ar")


def _patch_act_tables():
    """Make natural_log_exp_and_others the only provider of Exp/Ln so the
    table-load pass emits a single ACT_TABLE_LOAD instead of thrashing
    between the exp-only and ln-only sets. Set ids stay file-ordered."""
    import concourse.hw_specs as hw_specs

    orig = hw_specs.get_activation_tables
    if getattr(orig, "_arcface_patched", False):
        return

    def patched(arch):
        tabs = {k: set(v) for k, v in orig(arch).items()}
        for name, fns in tabs.items():
            if name != "natural_log_exp_and_others":
                fns.discard(mybir.ActivationFunctionType.Exp)
                fns.discard(mybir.ActivationFunctionType.Ln)
        return tabs

    patched._arcface_patched = True
    hw_specs.get_activation_tables = patched
    bacc.get_activation_tables = patched


def build_nc():
    _patch_act_tables()
    nc = bacc.Bacc(None)
    cos_p = nc.declare_dram_parameter("cosine", [B, CS], F32, isOutput=False)
    gidx_p = nc.declare_dram_parameter("gidx", [P, RT], I32, isOutput=False)
    own_p = nc.declare_dram_parameter("own", [P, RT], F32, isOutput=False)
    out_p = nc.declare_dram_parameter("out", [1, 1], F32, isOutput=True)

    cos_flat = cos_p[:].rearrange("b (c o) -> (b c) o", o=1)

    with tile.TileContext(nc) as tc:
        with (
            tc.tile_pool(name="data", bufs=3) as data_pool,
            tc.tile_pool(name="expp", bufs=2) as exp_pool,
            tc.tile_pool(name="small", bufs=1) as small,
            tc.tile_pool(name="dram", bufs=1, space="DRAM") as dram,
        ):
            # bias operand for exp(S*x - SHIFT) activations
            nbias = small.tile([P, 1], F32)
            nc.gpsimd.memset(nbias[:], -SHIFT)
            # dummy activation: pulls the ACT table load to the start of the
            # kernel instead of gating the first streaming exp
            warm_act = small.tile([P, 1], F32)
            nc.scalar.activation(
                out=warm_act[:], in_=nbias[:], func=mybir.ActivationFunctionType.Exp
            )

            # ---- warmup collective: zeros AllReduce issued first so the
            # one-time comm alignment overlaps the streaming pass
            warm_zero = None
            if WARMUP_CC:
                warm_sb = small.tile([P, 1], F32)
                nc.gpsimd.memset(warm_sb[:], 0.0)
                warm_in = dram.tile([P, 1], F32)
                warm_out = dram.tile([P, 1], F32)
                nc.sync.dma_start(out=warm_in[:], in_=warm_sb[:])
                nc.gpsimd.collective_compute(
                    "AllReduce",
                    mybir.AluOpType.add,
                    replica_groups=[list(range(NCORES))],
                    ins=[warm_in.opt()],
                    outs=[warm_out.opt()],
                )
                warm_zero = small.tile([P, 1], F32)
                nc.sync.dma_start(out=warm_zero[:], in_=warm_out[:])

            # ---- main streaming pass: exp + row-sum accumulate
            # First chunks are small so the ACT pipeline starts early: with
            # several DMAs in flight the SDMA engines round-robin at packet
            # granularity and equal-size chunks would all complete together.
            plans = [RAMP_PLAN] + [[FC] * NCH] * (RT - 1)
            assert all(sum(p) == CS for p in plans)
            fc_max = max(max(p) for p in plans)
            tcols = []  # per row tile: (start, count) in sums
            ncols = 0
            for t in range(RT):
                tcols.append((ncols, len(plans[t])))
                ncols += len(plans[t])
            sums = small.tile([P, ncols], F32)
            for t in range(RT):
                off = 0
                for i, w in enumerate(plans[t]):
                    dt = data_pool.tile([P, fc_max], F32, tag="data")
                    nc.sync.dma_start(
                        out=dt[:, 0:w],
                        in_=cos_p[t * P : (t + 1) * P, off : off + w],
                    )
                    ev = exp_pool.tile([P, fc_max], F32, tag="exp")
                    col = tcols[t][0] + i
                    nc.scalar.activation(
                        out=ev[:, 0:w], in_=dt[:, 0:w],
                        func=mybir.ActivationFunctionType.Exp,
                        scale=S, bias=nbias[:],
                        accum_out=sums[:, col : col + 1],
                    )
                    off += w

            # ---- gather target elements: idx in SBUF -> indirect DMA
            idx_sb = small.tile([P, RT], I32)
            own_sb = small.tile([P, RT], F32)
            gc = small.tile([P, RT], F32)  # gathered cosine at target cols
            nc.gpsimd.dma_start(out=idx_sb[:], in_=gidx_p[:])
            nc.gpsimd.dma_start(out=own_sb[:], in_=own_p[:])
            for t in range(RT):
                nc.gpsimd.indirect_dma_start(
                    out=gc[:, t : t + 1],
                    out_offset=None,
                    in_=cos_flat,
                    in_offset=bass.IndirectOffsetOnAxis(ap=idx_sb[:, t : t + 1], axis=0),
                )

            # ---- margin math on [128, RT] tensors
            cc_sb = small.tile([P, 2 * RT], F32)  # cols 0:RT sumexp, RT:2RT tgt
            spart = cc_sb[:, 0:RT]
            tpart = cc_sb[:, RT : 2 * RT]

            c = small.tile([P, RT], F32)
            nc.vector.tensor_scalar(
                out=c[:], in0=gc[:], scalar1=1.0 - EPS, scalar2=-1.0 + EPS,
                op0=mybir.AluOpType.min, op1=mybir.AluOpType.max,
            )
            # om = 1 - c^2  (via (c*c)*-1 + 1)
            om = small.tile([P, RT], F32)
            nc.vector.tensor_tensor(out=om[:], in0=c[:], in1=c[:], op=mybir.AluOpType.mult)
            nc.vector.tensor_scalar(
                out=om[:], in0=om[:], scalar1=-1.0, scalar2=1.0,
                op0=mybir.AluOpType.mult, op1=mybir.AluOpType.add,
            )
            # sine = exp(0.5 * ln(om)) — avoids the low-precision Sqrt table
            sine = small.tile([P, RT], F32)
            nc.scalar.activation(out=sine[:], in_=om[:], func=mybir.ActivationFunctionType.Ln)
            nc.scalar.activation(
                out=sine[:], in_=sine[:], func=mybir.ActivationFunctionType.Exp, scale=0.5
            )
            # phi = c*COS_M - sine*SIN_M
            phi = small.tile([P, RT], F32)
            t1 = small.tile([P, RT], F32)
            nc.vector.tensor_scalar(out=t1[:], in0=sine[:], scalar1=SIN_M, scalar2=None, op0=mybir.AluOpType.mult)
            nc.vector.scalar_tensor_tensor(
                out=phi[:], in0=c[:], scalar=COS_M, in1=t1[:],
                op0=mybir.AluOpType.mult, op1=mybir.AluOpType.subtract,
            )
            # phi = where(c > TH, phi, c - MM)
            gt = small.tile([P, RT], F32)
            nc.vector.tensor_scalar(out=gt[:], in0=c[:], scalar1=TH, scalar2=None, op0=mybir.AluOpType.is_gt)
            cmm = small.tile([P, RT], F32)
            nc.vector.tensor_scalar(out=cmm[:], in0=c[:], scalar1=MM, scalar2=None, op0=mybir.AluOpType.subtract)
            d = small.tile([P, RT], F32)
            nc.vector.tensor_tensor(out=d[:], in0=phi[:], in1=cmm[:], op=mybir.AluOpType.subtract)
            nc.vector.tensor_tensor(out=d[:], in0=d[:], in1=gt[:], op=mybir.AluOpType.mult)
            nc.vector.tensor_tensor(out=phi[:], in0=cmm[:], in1=d[:], op=mybir.AluOpType.add)

            # tpart = own * phi * S
            nc.vector.tensor_tensor(out=tpart, in0=own_sb[:], in1=phi[:], op=mybir.AluOpType.mult)
            nc.vector.tensor_scalar(out=tpart, in0=tpart, scalar1=S, scalar2=None, op0=mybir.AluOpType.mult)

            # delta = own * (exp(S*phi - SHIFT) - exp(S*c - SHIFT))
            e_phi = small.tile([P, RT], F32)
            e_c = small.tile([P, RT], F32)
            nc.scalar.activation(
                out=e_phi[:], in_=phi[:], func=mybir.ActivationFunctionType.Exp,
                scale=S, bias=nbias[:],
            )
            nc.scalar.activation(
                out=e_c[:], in_=c[:], func=mybir.ActivationFunctionType.Exp,
                scale=S, bias=nbias[:],
            )
            delta = small.tile([P, RT], F32)
            nc.vector.tensor_tensor(out=delta[:], in0=e_phi[:], in1=e_c[:], op=mybir.AluOpType.subtract)
            nc.vector.tensor_tensor(out=delta[:], in0=delta[:], in1=own_sb[:], op=mybir.AluOpType.mult)


            # spart[:, t] = sum_k sums[:, tcols[t]] + delta[:, t]
            red = small.tile([P, RT], F32)
            for t in range(RT):
                lo, n = tcols[t]
                nc.vector.tensor_reduce(
                    out=red[:, t : t + 1],
                    in_=sums[:, lo : lo + n],
                    axis=mybir.AxisListType.X,
                    op=mybir.AluOpType.add,
                )
            nc.vector.tensor_tensor(out=spart, in0=red[:], in1=delta[:], op=mybir.AluOpType.add)
            # consume the warmup output (all zeros) so it can't be DCE'd
            if WARMUP_CC:
                nc.vector.scalar_tensor_tensor(
                    out=spart[:, 0:1], in0=warm_zero[:], scalar=1.0, in1=spart[:, 0:1],
                    op0=mybir.AluOpType.mult, op1=mybir.AluOpType.add,
                )

            # ---- cross-core reduction of the [128, 2*RT] stats
            cc_in = dram.tile([P, 2 * RT], F32)
            nc.sync.dma_start(out=cc_in[:], in_=cc_sb[:])
            red_sb = small.tile([P, 2 * RT], F32)
            if CC_KIND == "ag":
                # AllGather (floor ~2x cheaper than AllReduce) + local reduce
                cc_gath = dram.tile([NCORES * P, 2 * RT], F32)
                nc.gpsimd.collective_compute(
                    "AllGather",
                    mybir.AluOpType.bypass,
                    replica_groups=[list(range(NCORES))],
                    ins=[cc_in.opt()],
                    outs=[cc_gath.opt()],
                )
                gath_sb = small.tile([P, NCORES * 2 * RT], F32)
                nc.sync.dma_start(
                    out=gath_sb[:],
                    in_=cc_gath[:].rearrange("(r p) j -> p r j", p=P),
                )
                nc.vector.tensor_reduce(
                    out=red_sb[:],
                    in_=gath_sb[:].rearrange("p (r j) -> p j r", r=NCORES),
                    axis=mybir.AxisListType.X,
                    op=mybir.AluOpType.add,
                )
            else:
                cc_out = dram.tile([P, 2 * RT], F32)
                nc.gpsimd.collective_compute(
                    "AllReduce",
                    mybir.AluOpType.add,
                    replica_groups=[list(range(NCORES))],
                    ins=[cc_in.opt()],
                    outs=[cc_out.opt()],
                )
                nc.sync.dma_start(out=red_sb[:], in_=cc_out[:])

            # ---- loss = SHIFT + mean(log(sumexp) - tgt)
            logs = small.tile([P, RT], F32)
            nc.scalar.activation(
                out=logs[:], in_=red_sb[:, 0:RT], func=mybir.ActivationFunctionType.Ln
            )
            lvec = small.tile([P, RT], F32)
            nc.vector.tensor_tensor(
                out=lvec[:], in0=logs[:], in1=red_sb[:, RT : 2 * RT],
                op=mybir.AluOpType.subtract,
            )
            lrow = small.tile([P, 1], F32)
            nc.vector.tensor_reduce(
                out=lrow[:], in_=lvec[:], axis=mybir.AxisListType.X, op=mybir.AluOpType.add
            )
            ltot = small.tile([P, 1], F32)
            nc.gpsimd.partition_all_reduce(
                ltot[:], lrow[:], channels=P, reduce_op=bass_isa.ReduceOp.add
            )
            res = small.tile([1, 1], F32)
            nc.scalar.activation(
                out=res[:], in_=ltot[0:1, :], func=mybir.ActivationFunctionType.Copy,
                scale=1.0 / B, bias=SHIFT,
            )
            nc.sync.dma_start(out=out_p[:], in_=res[:])

    nc.finalize()
    return nc


_CACHE = {}


def _get_nc():
    if "nc" not in _CACHE:
        _CACHE["nc"] = build_nc()
    return _CACHE["nc"]


def make_in_maps(cosine: np.ndarray, labels: np.ndarray):
    labels = np.asarray(labels).astype(np.int64)
    rows = np.arange(B, dtype=np.int64)
    in_maps = []
    for m in range(NCORES):
        lo = m * CS
        owned = (labels >= lo) & (labels < lo + CS)
        local = np.where(owned, labels - lo, 0)
        gidx = np.ascontiguousarray((rows * CS + local).astype(np.int32).reshape(RT, P).T)
        own = np.ascontiguousarray(owned.astype(np.float32).reshape(RT, P).T)
        shard = np.ascontiguousarray(cosine[:, lo : lo + CS], dtype=np.float32)
        in_maps.append({"cosine": shard, "gidx": gidx, "own": own})
    return in_maps


def kernel(cosine: np.ndarray, labels: np.ndarray, _trace: bool = False):
    nc = _get_nc()
    in_maps = make_in_maps(np.asarray(cosine, dtype=np.float32), labels)
    res = run_bass_kernel_spmd(
        nc, in_maps, core_ids=list(range(NCORES)), trace=_trace
    )
    out = np.asarray(res.results[0]["out"], dtype=np.float32).reshape(())
    if _trace:
        return out, res
    return out
